# revision 13
# baseline (speedup 1.0000x reference)
"""Trainium2 Bass kernel for nn_DirectInjectionEncoder (moe_routing).

Strategy (8 NeuronCores), v4:
  - The three big projection GEMMs (Wgate/Wup/Wdown, 10240->2560) are
    sharded over the output dim d_model=2560 -> 320 columns per core; each
    core streams 1/8 of the big weights and computes its 320-column slice
    of all 16*36=576 rows per group.
  - Row L2-norms need the full 2560-dim row: each core computes partial
    sums of squares for its slice; ONE tiny 8-core AllReduce(add) per big
    group fires as soon as that group's GEMM drains, so norm readback +
    scaling + stores overlap the next group's GEMM. Only the last group's
    AllReduce sits near the tail, and its ~20us latency is covered by the
    kv projections and identity tokens scheduled last.
  - The small projections (Wk/Wv, 640->2560) are data-parallel over the
    batch (weights replicated, norms core-local) and run at the very end,
    inside the last AllReduce's latency window.
  - Identity tokens (9 of 14 slots/layer, first 2560 dims, no weights) are
    data-parallel over the batch: core c handles batches [2c, 2c+1],
    streamed in/out as bf16. Tiles 0-1 run early (engine warmup), 2-5 in
    the tail window.
  - All GEMM operands and outputs are bf16 (PSUM accumulation and norm
    math in f32); the host converts outputs back to f32. Sum-of-squares
    runs on DVE (tensor_mul + reduce_sum).
  - NOTE: no readback of the warmup-collective result. A rank-1
    dram->sbuf readback of warm_out[0, :] lowers to a corrupt
    partition-strided descriptor (stride = partition_pitch - 60) that
    scribbles zeros over unrelated SBUF tiles. Nothing consumes the
    warmup result, so it is simply not read back.
"""

import os
import sys

sys.path.insert(0, "/opt/trn_rl_repo")

import numpy as np
import ml_dtypes

from concourse import bacc, bass, mybir
from concourse.bass_utils import run_bass_kernel_spmd
from concourse.tile import TileContext

D_MODEL = 2560
NUM_LAYERS = 36
TOKENS_PER_LAYER = 14
B = 16
N_CORES = 8
CORE_IDS = list(range(N_CORES))
D_SHARD = D_MODEL // N_CORES  # 320
ROWS = B * NUM_LAYERS  # 576
ROWS_PC = ROWS // N_CORES  # 72 rows/core for the batch-parallel paths
ROW_TILES = [(0, 128), (128, 128), (256, 128), (384, 128), (512, 64)]
NRT = len(ROW_TILES)

IDENTITY_OFFSETS = np.array([0, 1, 2, 4, 6, 7, 8, 10, 13])
BIG_GROUPS = [(11, "Wup"), (9, "Wgate"), (12, "Wdown")]  # in_dim 10240, d-sharded
KV_GROUPS = [(3, "Wk"), (5, "Wv")]  # in_dim 640, batch-parallel
KV_IND = 640
BIG_IND = 10240
KB_BIG = 4  # k-tiles per DMA super-tile
N_CHUNKS = D_MODEL // 512  # 5 psum chunks for the kv path

ID_ROWS = (B // N_CORES) * NUM_LAYERS * len(IDENTITY_OFFSETS)  # 648
ID_TILES = [(0, 128), (128, 128), (256, 128), (384, 128), (512, 128), (640, 8)]
N_SSQ_COLS = len(BIG_GROUPS) * NRT  # 15

F32 = mybir.dt.float32
BF16 = mybir.dt.bfloat16
NP_BF16 = ml_dtypes.bfloat16
AF = mybir.ActivationFunctionType
MUL = mybir.AluOpType.mult


def _positions(offset):
    return np.arange(NUM_LAYERS) * TOKENS_PER_LAYER + offset


def build_program():
    nc = bacc.Bacc("TRN2", num_devices=N_CORES)

    xt_d, wt_d, om_d = [], [], []
    nsup_big = BIG_IND // (128 * KB_BIG)
    for gi, (off, wname) in enumerate(BIG_GROUPS):
        xt_d.append(nc.declare_dram_parameter(f"xt_{gi}", [nsup_big, 128, KB_BIG * ROWS], BF16, isOutput=False))
        wt_d.append(nc.declare_dram_parameter(f"wt_{gi}", [nsup_big, 128, KB_BIG * D_SHARD], BF16, isOutput=False))
        om_d.append(nc.declare_dram_parameter(f"om_{gi}", [ROWS, D_SHARD], BF16, isOutput=True))
    kvx_d, kvw_d, kvo_d = [], [], []
    for gi, (off, wname) in enumerate(KV_GROUPS):
        kvx_d.append(nc.declare_dram_parameter(f"kvx_{gi}", [128, 5 * ROWS_PC], BF16, isOutput=False))
        kvw_d.append(nc.declare_dram_parameter(f"kvw_{gi}", [128, 5 * D_MODEL], BF16, isOutput=False))
        kvo_d.append(nc.declare_dram_parameter(f"kvo_{gi}", [ROWS_PC, D_MODEL], BF16, isOutput=True))
    idx_d = nc.declare_dram_parameter("id_x", [ID_ROWS, D_MODEL], BF16, isOutput=False)
    ido_d = nc.declare_dram_parameter("out_id", [ID_ROWS, D_MODEL], BF16, isOutput=True)

    with TileContext(nc) as tc:
        with (
            tc.tile_pool(name="xt", bufs=6) as xt_pool,
            tc.tile_pool(name="wt", bufs=6) as wt_pool,
            tc.tile_pool(name="sout", bufs=N_SSQ_COLS) as sout_pool,
            tc.tile_pool(name="scr", bufs=2) as scr_pool,
            tc.tile_pool(name="kvp", bufs=2) as kv_pool,
            tc.tile_pool(name="idp", bufs=6) as id_pool,
            tc.tile_pool(name="idscr", bufs=2) as idscr_pool,
            tc.tile_pool(name="small", bufs=1) as small_pool,
            tc.tile_pool(name="ps", bufs=8, space="PSUM") as psum_pool,
            tc.tile_pool(name="dram", bufs=1, space="DRAM") as dram_pool,
        ):
            ssq = small_pool.tile([128, N_SSQ_COLS], F32, tag="ssq")
            nc.vector.memset(ssq[:], 0.0)

            # Warmup collective: the first collective in a NEFF pays ~60us
            # of one-time setup; fire a tiny dummy at kernel start so the
            # real per-group AllReduces only pay marginal latency.
            warm_sb = small_pool.tile([1, 16], F32, tag="warmsb")
            nc.vector.memset(warm_sb[:], 0.0)
            warm_in = dram_pool.tile([16], F32, tag="warmci")
            warm_out = dram_pool.tile([N_CORES, 16], F32, tag="warmco")
            nc.gpsimd.dma_start(out=warm_in[:], in_=warm_sb[0, :])
            nc.gpsimd.collective_compute(
                "AllGather",
                mybir.AluOpType.bypass,
                ins=[warm_in.opt()],
                outs=[warm_out.opt()],
                replica_groups=[CORE_IDS],
            )
            # (no readback -- see NOTE in the module docstring)

            # ---- identity tokens: loads on the SP ring, compute split ----
            id_tiles = {}

            def identity_load(t):
                t0, tw = ID_TILES[t]
                it = id_pool.tile([128, D_MODEL], BF16, tag="idp", name=f"idp_{t}")
                nc.sync.dma_start(out=it[:tw, :], in_=idx_d[t0 : t0 + tw, :])
                id_tiles[t] = it

            def identity_compute(t):
                t0, tw = ID_TILES[t]
                it = id_tiles[t]
                iscr = idscr_pool.tile([128, D_MODEL], BF16, tag="idscr", name=f"idscr_{t}")
                issq = small_pool.tile([128, 1], F32, tag=f"idssq{t}", name=f"idssq_{t}")
                nc.scalar.activation(
                    iscr[:tw, :], it[:tw, :], AF.Square,
                    accum_out=issq[:tw, :],
                )
                inorm = small_pool.tile([128, 1], F32, tag=f"idnorm{t}", name=f"idnorm_{t}")
                nc.scalar.sqrt(inorm[:tw, :], issq[:tw, :])
                iscale = small_pool.tile([128, 1], F32, tag=f"idscale{t}", name=f"idscale_{t}")
                nc.vector.reciprocal(iscale[:tw, :], inorm[:tw, :])
                if t % 2 == 0:
                    nc.vector.tensor_scalar_mul(it[:tw, :], it[:tw, :], iscale[:tw, :])
                else:
                    nc.scalar.activation(it[:tw, :], it[:tw, :], AF.Copy, scale=iscale[:tw, :])
                nc.scalar.dma_start(out=ido_d[t0 : t0 + tw, :], in_=it[:tw, :])

            # ---- kv operand loads (SP ring), emitted early to prefetch ----
            kv_tiles = {}

            def kv_load(gi):
                kvx = kv_pool.tile([128, 5, ROWS_PC], BF16, tag="kvx", name=f"kvx_{gi}")
                kvw = kv_pool.tile([128, 5, D_MODEL], BF16, tag="kvw", name=f"kvw_{gi}")
                nc.sync.dma_start(
                    out=kvx[:], in_=kvx_d[gi].rearrange("p (k c) -> p k c", k=5)
                )
                nc.sync.dma_start(
                    out=kvw[:], in_=kvw_d[gi].rearrange("p (k c) -> p k c", k=5)
                )
                kv_tiles[gi] = (kvx, kvw)

            # ---- big groups: d-sharded GEMMs, PSUM-accumulated over k ----
            souts = {}

            def gemm_group(gi):
                nk = BIG_IND // 128
                ps = [
                    psum_pool.tile([128, D_SHARD], F32, tag="ps", name=f"ps_{gi}_{ri}")
                    for ri in range(NRT)
                ]
                xt_view = xt_d[gi].rearrange("j p (kb c) -> j p kb c", kb=KB_BIG)
                wt_view = wt_d[gi].rearrange("j p (kb c) -> j p kb c", kb=KB_BIG)
                for j in range(nsup_big):
                    xt = xt_pool.tile([128, KB_BIG, ROWS], BF16, tag="xt", name=f"xt_{gi}_{j}")
                    wt = wt_pool.tile([128, KB_BIG, D_SHARD], BF16, tag="wt", name=f"wt_{gi}_{j}")
                    nc.sync.dma_start(out=xt[:], in_=xt_view[j])
                    nc.sync.dma_start(out=wt[:], in_=wt_view[j])
                    for k in range(KB_BIG):
                        kt = j * KB_BIG + k
                        for r, (r0, rw) in enumerate(ROW_TILES):
                            nc.tensor.matmul(
                                ps[r][:rw, :],
                                xt[:, k, r0 : r0 + rw],
                                wt[:, k, :],
                                start=(kt == 0),
                                stop=(kt == nk - 1),
                            )
                for r, (r0, rw) in enumerate(ROW_TILES):
                    col = gi * NRT + r
                    so = sout_pool.tile([128, D_SHARD], BF16, tag="sout", name=f"so_{gi}_{r}")
                    scr = scr_pool.tile([128, D_SHARD], BF16, tag="scr", name=f"scr_{gi}_{r}")
                    nc.vector.tensor_copy(so[:rw, :], ps[r][:rw, :])
                    nc.vector.tensor_mul(scr[:rw, :], ps[r][:rw, :], so[:rw, :])
                    nc.vector.reduce_sum(
                        ssq[:rw, col : col + 1], scr[:rw, :], axis=mybir.AxisListType.X
                    )
                    souts[(gi, r)] = so

            # Per-group AllReduce of ssq partials + norms + scale + store.
            def finish_group(gi):
                c0, c1 = gi * NRT, (gi + 1) * NRT
                cc_in = dram_pool.tile([128, NRT], F32, tag=f"ccin{gi}", name=f"ccin_{gi}")
                cc_out = dram_pool.tile([128, NRT], F32, tag=f"ccout{gi}", name=f"ccout_{gi}")
                nc.gpsimd.dma_start(out=cc_in[:], in_=ssq[:, c0:c1])
                nc.gpsimd.collective_compute(
                    "AllReduce",
                    mybir.AluOpType.add,
                    ins=[cc_in.opt()],
                    outs=[cc_out.opt()],
                    replica_groups=[CORE_IDS],
                )
                tsq = small_pool.tile([128, NRT], F32, tag=f"tsq{gi}", name=f"tsq_{gi}")
                nc.gpsimd.dma_start(out=tsq[:], in_=cc_out[:])
                nc.scalar.sqrt(tsq[:], tsq[:])
                scale = small_pool.tile([128, NRT], F32, tag=f"scale{gi}", name=f"scale_{gi}")
                nc.vector.reciprocal(scale[:], tsq[:])
                for r, (r0, rw) in enumerate(ROW_TILES):
                    so = souts[(gi, r)]
                    if r % 2 == 0:
                        nc.vector.tensor_scalar_mul(
                            so[:rw, :], so[:rw, :], scale[:rw, r : r + 1]
                        )
                    else:
                        nc.scalar.activation(
                            so[:rw, :], so[:rw, :], AF.Copy,
                            scale=scale[:rw, r : r + 1],
                        )
                    nc.scalar.dma_start(out=om_d[gi][r0 : r0 + rw, :], in_=so[:rw, :])

            # ---- kv groups: batch-parallel GEMMs with core-local norms ----
            def kv_group(gi):
                kvx, kvw = kv_tiles[gi]
                pcs = [
                    psum_pool.tile([128, 512], F32, tag="ps", name=f"pkv_{gi}_{ci}")
                    for ci in range(N_CHUNKS)
                ]
                for k in range(5):
                    for ci in range(N_CHUNKS):
                        nc.tensor.matmul(
                            pcs[ci][:ROWS_PC, :],
                            kvx[:, k, :],
                            kvw[:, k, ci * 512 : (ci + 1) * 512],
                            start=(k == 0),
                            stop=(k == 4),
                        )
                kvo = kv_pool.tile([128, D_MODEL], BF16, tag="kvo", name=f"kvo_{gi}")
                kvssq = small_pool.tile([128, N_CHUNKS], F32, tag=f"kvssq{gi}", name=f"kvssq_{gi}")
                for ci in range(N_CHUNKS):
                    sl = slice(ci * 512, (ci + 1) * 512)
                    kscr = kv_pool.tile([128, 512], BF16, tag="kscr", name=f"kscr_{gi}_{ci}")
                    nc.vector.tensor_copy(kvo[:ROWS_PC, sl], pcs[ci][:ROWS_PC, :])
                    nc.vector.tensor_mul(kscr[:ROWS_PC, :], pcs[ci][:ROWS_PC, :], kvo[:ROWS_PC, sl])
                    nc.vector.reduce_sum(
                        kvssq[:ROWS_PC, ci : ci + 1], kscr[:ROWS_PC, :], axis=mybir.AxisListType.X
                    )
                kvs = small_pool.tile([128, 1], F32, tag=f"kvs{gi}", name=f"kvs_{gi}")
                nc.vector.reduce_sum(kvs[:ROWS_PC, :], kvssq[:ROWS_PC, :], axis=mybir.AxisListType.X)
                nc.scalar.sqrt(kvs[:ROWS_PC, :], kvs[:ROWS_PC, :])
                kvsc = small_pool.tile([128, 1], F32, tag=f"kvsc{gi}", name=f"kvsc_{gi}")
                nc.vector.reciprocal(kvsc[:ROWS_PC, :], kvs[:ROWS_PC, :])
                nc.vector.tensor_scalar_mul(kvo[:ROWS_PC, :], kvo[:ROWS_PC, :], kvsc[:ROWS_PC, :])
                nc.sync.dma_start(out=kvo_d[gi][:, :], in_=kvo[:ROWS_PC, :])

            # ---- schedule ----
            identity_load(0)
            gemm_group(0)          # Wup
            identity_compute(0)
            identity_load(1)
            identity_compute(1)
            gemm_group(1)          # Wgate
            finish_group(0)        # AllReduce A overlaps Wgate/Wdown
            kv_load(0)
            kv_load(1)
            identity_load(2)
            identity_load(3)
            gemm_group(2)          # Wdown
            finish_group(1)        # AllReduce B overlaps Wdown
            identity_load(4)
            identity_load(5)
            kv_group(0)            # tail work: covers AllReduce C latency
            kv_group(1)
            identity_compute(2)
            identity_compute(3)
            identity_compute(4)
            identity_compute(5)
            finish_group(2)        # AllReduce C + final scale/store

    nc.compile()
    return nc


_NC = None


def _get_nc():
    global _NC
    if _NC is None:
        _NC = build_program()
    return _NC


def _prep_inputs(lora_tokens, weights):
    """Host-side sharding: gather token groups, transpose contraction onto
    partitions, slice weights per core, bf16-ify."""
    lora = np.ascontiguousarray(lora_tokens)

    def pack_supertiles(arr_t, kb):
        # [K, C] -> [K/(128*kb), 128, kb*C]: dense per-partition runs so each
        # super-tile DMA is one fully-contiguous block.
        K, C = arr_t.shape
        nsup = K // (128 * kb)
        return np.ascontiguousarray(
            arr_t.reshape(nsup, kb, 128, C).transpose(0, 2, 1, 3).reshape(nsup, 128, kb * C)
        )

    def pack_kv(arr_t):
        # [640, C] -> [128, 5*C]
        K, C = arr_t.shape
        return np.ascontiguousarray(
            arr_t.reshape(5, 128, C).transpose(1, 0, 2).reshape(128, 5 * C)
        )

    shared = {}
    for gi, (off, wname) in enumerate(BIG_GROUPS):
        pos = _positions(off)
        x = lora[:, pos, :].reshape(ROWS, BIG_IND)
        shared[f"xt_{gi}"] = pack_supertiles(x.T.astype(NP_BF16), KB_BIG)
    kv_x = {}
    for gi, (off, wname) in enumerate(KV_GROUPS):
        pos = _positions(offset=off)
        kv_x[gi] = lora[:, pos, :KV_IND].reshape(ROWS, KV_IND)
        shared[f"kvw_{gi}"] = pack_kv(weights[wname].T.astype(NP_BF16))

    id_pos = np.sort(np.concatenate([_positions(o) for o in IDENTITY_OFFSETS]))
    in_maps = []
    bpc = B // N_CORES
    for c in range(N_CORES):
        m = dict(shared)
        for gi, (off, wname) in enumerate(BIG_GROUPS):
            wsl = weights[wname][c * D_SHARD : (c + 1) * D_SHARD, :]  # [320, 10240]
            m[f"wt_{gi}"] = pack_supertiles(wsl.T.astype(NP_BF16), KB_BIG)
        for gi in range(len(KV_GROUPS)):
            m[f"kvx_{gi}"] = pack_kv(
                kv_x[gi][c * ROWS_PC : (c + 1) * ROWS_PC, :].T.astype(NP_BF16)
            )
        m["id_x"] = np.ascontiguousarray(
            lora[c * bpc : (c + 1) * bpc, :, :][:, id_pos, :D_MODEL]
        ).reshape(ID_ROWS, D_MODEL).astype(NP_BF16)
        in_maps.append(m)
    return in_maps, id_pos


def run(inputs, trace=False):
    nc = _get_nc()
    weights = {k: inputs[k] for k in ("Wk", "Wv", "Wgate", "Wup", "Wdown")}
    in_maps, id_pos = _prep_inputs(inputs["lora_tokens"], weights)
    res = run_bass_kernel_spmd(nc, in_maps, CORE_IDS, trace=trace)

    out = np.zeros((B, NUM_LAYERS * TOKENS_PER_LAYER, D_MODEL), dtype=np.float32)
    bpc = B // N_CORES
    for c in range(N_CORES):
        r = res.results[c]
        out[c * bpc : (c + 1) * bpc, id_pos, :] = (
            r["out_id"].astype(np.float32).reshape(bpc, len(id_pos), D_MODEL)
        )
        for gi, (off, wname) in enumerate(BIG_GROUPS):
            pos = _positions(off)
            out[:, pos, c * D_SHARD : (c + 1) * D_SHARD] = (
                r[f"om_{gi}"].astype(np.float32).reshape(B, NUM_LAYERS, D_SHARD)
            )
        for gi, (off, wname) in enumerate(KV_GROUPS):
            pos = _positions(off)
            out[c * bpc : (c + 1) * bpc, pos, :] = (
                r[f"kvo_{gi}"].astype(np.float32).reshape(bpc, NUM_LAYERS, D_MODEL)
            )
    return out, res


def kernel(**inputs) -> np.ndarray:
    out, _ = run(inputs, trace=False)
    return out


# revision 16
# speedup vs baseline: 1.0327x; 1.0327x over previous
"""Trainium2 Bass kernel for nn_DirectInjectionEncoder (moe_routing).

Strategy (8 NeuronCores), v4:
  - The three big projection GEMMs (Wgate/Wup/Wdown, 10240->2560) are
    sharded over the output dim d_model=2560 -> 320 columns per core; each
    core streams 1/8 of the big weights and computes its 320-column slice
    of all 16*36=576 rows per group.
  - Row L2-norms need the full 2560-dim row: each core computes partial
    sums of squares for its slice; ONE tiny 8-core AllReduce(add) per big
    group fires as soon as that group's GEMM drains, so norm readback +
    scaling + stores overlap the next group's GEMM. Only the last group's
    AllReduce sits near the tail, and its ~20us latency is covered by the
    kv projections and identity tokens scheduled last.
  - The small projections (Wk/Wv, 640->2560) are data-parallel over the
    batch (weights replicated, norms core-local) and run at the very end,
    inside the last AllReduce's latency window.
  - Identity tokens (9 of 14 slots/layer, first 2560 dims, no weights) are
    data-parallel over the batch: core c handles batches [2c, 2c+1],
    streamed in/out as bf16. Tiles 0-1 run early (engine warmup), 2-5 in
    the tail window.
  - All GEMM operands and outputs are bf16 (PSUM accumulation and norm
    math in f32); the host converts outputs back to f32. Sum-of-squares
    runs on DVE (tensor_mul + reduce_sum).
  - NOTE: no readback of the warmup-collective result. A rank-1
    dram->sbuf readback of warm_out[0, :] lowers to a corrupt
    partition-strided descriptor (stride = partition_pitch - 60) that
    scribbles zeros over unrelated SBUF tiles. Nothing consumes the
    warmup result, so it is simply not read back.
"""

import os
import sys

sys.path.insert(0, "/opt/trn_rl_repo")

import numpy as np
import ml_dtypes

from concourse import bacc, bass, mybir
from concourse.bass_utils import run_bass_kernel_spmd
from concourse.tile import TileContext

D_MODEL = 2560
NUM_LAYERS = 36
TOKENS_PER_LAYER = 14
B = 16
N_CORES = 8
CORE_IDS = list(range(N_CORES))
D_SHARD = D_MODEL // N_CORES  # 320
ROWS = B * NUM_LAYERS  # 576
ROWS_PC = ROWS // N_CORES  # 72 rows/core for the batch-parallel paths
ROW_TILES = [(0, 128), (128, 128), (256, 128), (384, 128), (512, 64)]
NRT = len(ROW_TILES)

IDENTITY_OFFSETS = np.array([0, 1, 2, 4, 6, 7, 8, 10, 13])
BIG_GROUPS = [(11, "Wup"), (9, "Wgate"), (12, "Wdown")]  # in_dim 10240, d-sharded
KV_GROUPS = [(3, "Wk"), (5, "Wv")]  # in_dim 640, batch-parallel
KV_IND = 640
BIG_IND = 10240
KB_BIG = 4  # k-tiles per DMA super-tile
N_CHUNKS = D_MODEL // 512  # 5 psum chunks for the kv path

ID_ROWS = (B // N_CORES) * NUM_LAYERS * len(IDENTITY_OFFSETS)  # 648
ID_TILES = [(0, 128), (128, 128), (256, 128), (384, 128), (512, 128), (640, 8)]
N_SSQ_COLS = len(BIG_GROUPS) * NRT  # 15

F32 = mybir.dt.float32
BF16 = mybir.dt.bfloat16
NP_BF16 = ml_dtypes.bfloat16
AF = mybir.ActivationFunctionType
MUL = mybir.AluOpType.mult


def _positions(offset):
    return np.arange(NUM_LAYERS) * TOKENS_PER_LAYER + offset


def build_program():
    nc = bacc.Bacc("TRN2", num_devices=N_CORES)

    xt_d, wt_d, om_d = [], [], []
    nsup_big = BIG_IND // (128 * KB_BIG)
    for gi, (off, wname) in enumerate(BIG_GROUPS):
        xt_d.append(nc.declare_dram_parameter(f"xt_{gi}", [nsup_big, 128, KB_BIG * ROWS], BF16, isOutput=False))
        wt_d.append(nc.declare_dram_parameter(f"wt_{gi}", [nsup_big, 128, KB_BIG * D_SHARD], BF16, isOutput=False))
        om_d.append(nc.declare_dram_parameter(f"om_{gi}", [ROWS, D_SHARD], BF16, isOutput=True))
    kvx_d, kvw_d, kvo_d = [], [], []
    for gi, (off, wname) in enumerate(KV_GROUPS):
        kvx_d.append(nc.declare_dram_parameter(f"kvx_{gi}", [128, 5 * ROWS_PC], BF16, isOutput=False))
        kvw_d.append(nc.declare_dram_parameter(f"kvw_{gi}", [128, 5 * D_MODEL], BF16, isOutput=False))
        kvo_d.append(nc.declare_dram_parameter(f"kvo_{gi}", [ROWS_PC, D_MODEL], BF16, isOutput=True))
    idx_d = nc.declare_dram_parameter("id_x", [ID_ROWS, D_MODEL], BF16, isOutput=False)
    ido_d = nc.declare_dram_parameter("out_id", [ID_ROWS, D_MODEL], BF16, isOutput=True)

    with TileContext(nc) as tc:
        with (
            tc.tile_pool(name="xt", bufs=6) as xt_pool,
            tc.tile_pool(name="wt", bufs=6) as wt_pool,
            tc.tile_pool(name="sout", bufs=N_SSQ_COLS) as sout_pool,
            tc.tile_pool(name="scr", bufs=2) as scr_pool,
            tc.tile_pool(name="kvp", bufs=2) as kv_pool,
            tc.tile_pool(name="idp", bufs=6) as id_pool,
            tc.tile_pool(name="idscr", bufs=2) as idscr_pool,
            tc.tile_pool(name="small", bufs=1) as small_pool,
            tc.tile_pool(name="ps", bufs=8, space="PSUM") as psum_pool,
            tc.tile_pool(name="dram", bufs=1, space="DRAM") as dram_pool,
        ):
            ssq = small_pool.tile([128, N_SSQ_COLS], F32, tag="ssq")
            nc.vector.memset(ssq[:], 0.0)

            # Warmup collective: the first collective in a NEFF pays ~60us
            # of one-time setup; fire a tiny dummy at kernel start so the
            # real per-group AllReduces only pay marginal latency.
            warm_sb = small_pool.tile([1, 16], F32, tag="warmsb")
            nc.vector.memset(warm_sb[:], 0.0)
            warm_in = dram_pool.tile([16], F32, tag="warmci")
            warm_out = dram_pool.tile([N_CORES, 16], F32, tag="warmco")
            nc.gpsimd.dma_start(out=warm_in[:], in_=warm_sb[0, :])
            nc.gpsimd.collective_compute(
                "AllGather",
                mybir.AluOpType.bypass,
                ins=[warm_in.opt()],
                outs=[warm_out.opt()],
                replica_groups=[CORE_IDS],
            )
            # (no readback -- see NOTE in the module docstring)

            # ---- identity tokens: loads on the SP ring, compute split ----
            id_tiles = {}

            def identity_load(t):
                # Rides the ACT HWDGE queue so it never stalls the xt/wt
                # operand stream on the SP queue.
                t0, tw = ID_TILES[t]
                it = id_pool.tile([128, D_MODEL], BF16, tag="idp", name=f"idp_{t}")
                nc.scalar.dma_start(out=it[:tw, :], in_=idx_d[t0 : t0 + tw, :])
                id_tiles[t] = it

            def identity_compute(t):
                t0, tw = ID_TILES[t]
                it = id_tiles[t]
                iscr = idscr_pool.tile([128, D_MODEL], BF16, tag="idscr", name=f"idscr_{t}")
                issq = small_pool.tile([128, 1], F32, tag=f"idssq{t}", name=f"idssq_{t}")
                nc.scalar.activation(
                    iscr[:tw, :], it[:tw, :], AF.Square,
                    accum_out=issq[:tw, :],
                )
                inorm = small_pool.tile([128, 1], F32, tag=f"idnorm{t}", name=f"idnorm_{t}")
                nc.scalar.sqrt(inorm[:tw, :], issq[:tw, :])
                iscale = small_pool.tile([128, 1], F32, tag=f"idscale{t}", name=f"idscale_{t}")
                nc.vector.reciprocal(iscale[:tw, :], inorm[:tw, :])
                if t % 2 == 0:
                    nc.vector.tensor_scalar_mul(it[:tw, :], it[:tw, :], iscale[:tw, :])
                else:
                    nc.scalar.activation(it[:tw, :], it[:tw, :], AF.Copy, scale=iscale[:tw, :])
                nc.scalar.dma_start(out=ido_d[t0 : t0 + tw, :], in_=it[:tw, :])

            # ---- kv operand loads (SP ring), emitted early to prefetch ----
            kv_tiles = {}

            def kv_load(gi):
                # ACT HWDGE queue: see identity_load.
                kvx = kv_pool.tile([128, 5, ROWS_PC], BF16, tag="kvx", name=f"kvx_{gi}")
                kvw = kv_pool.tile([128, 5, D_MODEL], BF16, tag="kvw", name=f"kvw_{gi}")
                nc.scalar.dma_start(
                    out=kvx[:], in_=kvx_d[gi].rearrange("p (k c) -> p k c", k=5)
                )
                nc.scalar.dma_start(
                    out=kvw[:], in_=kvw_d[gi].rearrange("p (k c) -> p k c", k=5)
                )
                kv_tiles[gi] = (kvx, kvw)

            # ---- big groups: d-sharded GEMMs, PSUM-accumulated over k ----
            souts = {}

            def gemm_group(gi):
                nk = BIG_IND // 128
                ps = [
                    psum_pool.tile([128, D_SHARD], F32, tag="ps", name=f"ps_{gi}_{ri}")
                    for ri in range(NRT)
                ]
                xt_view = xt_d[gi].rearrange("j p (kb c) -> j p kb c", kb=KB_BIG)
                wt_view = wt_d[gi].rearrange("j p (kb c) -> j p kb c", kb=KB_BIG)
                for j in range(nsup_big):
                    xt = xt_pool.tile([128, KB_BIG, ROWS], BF16, tag="xt", name=f"xt_{gi}_{j}")
                    wt = wt_pool.tile([128, KB_BIG, D_SHARD], BF16, tag="wt", name=f"wt_{gi}_{j}")
                    nc.sync.dma_start(out=xt[:], in_=xt_view[j])
                    nc.sync.dma_start(out=wt[:], in_=wt_view[j])
                    for k in range(KB_BIG):
                        kt = j * KB_BIG + k
                        for r, (r0, rw) in enumerate(ROW_TILES):
                            nc.tensor.matmul(
                                ps[r][:rw, :],
                                xt[:, k, r0 : r0 + rw],
                                wt[:, k, :],
                                start=(kt == 0),
                                stop=(kt == nk - 1),
                            )
                for r, (r0, rw) in enumerate(ROW_TILES):
                    col = gi * NRT + r
                    so = sout_pool.tile([128, D_SHARD], BF16, tag="sout", name=f"so_{gi}_{r}")
                    scr = scr_pool.tile([128, D_SHARD], BF16, tag="scr", name=f"scr_{gi}_{r}")
                    nc.vector.tensor_copy(so[:rw, :], ps[r][:rw, :])
                    nc.vector.tensor_mul(scr[:rw, :], ps[r][:rw, :], so[:rw, :])
                    nc.vector.reduce_sum(
                        ssq[:rw, col : col + 1], scr[:rw, :], axis=mybir.AxisListType.X
                    )
                    souts[(gi, r)] = so

            # Per-group AllReduce of ssq partials + norms + scale + store.
            def finish_group(gi):
                c0, c1 = gi * NRT, (gi + 1) * NRT
                cc_in = dram_pool.tile([128, NRT], F32, tag=f"ccin{gi}", name=f"ccin_{gi}")
                cc_out = dram_pool.tile([128, NRT], F32, tag=f"ccout{gi}", name=f"ccout_{gi}")
                nc.gpsimd.dma_start(out=cc_in[:], in_=ssq[:, c0:c1])
                nc.gpsimd.collective_compute(
                    "AllReduce",
                    mybir.AluOpType.add,
                    ins=[cc_in.opt()],
                    outs=[cc_out.opt()],
                    replica_groups=[CORE_IDS],
                )
                tsq = small_pool.tile([128, NRT], F32, tag=f"tsq{gi}", name=f"tsq_{gi}")
                nc.gpsimd.dma_start(out=tsq[:], in_=cc_out[:])
                nc.scalar.sqrt(tsq[:], tsq[:])
                scale = small_pool.tile([128, NRT], F32, tag=f"scale{gi}", name=f"scale_{gi}")
                nc.vector.reciprocal(scale[:], tsq[:])
                for r, (r0, rw) in enumerate(ROW_TILES):
                    so = souts[(gi, r)]
                    if r % 2 == 0:
                        nc.vector.tensor_scalar_mul(
                            so[:rw, :], so[:rw, :], scale[:rw, r : r + 1]
                        )
                    else:
                        nc.scalar.activation(
                            so[:rw, :], so[:rw, :], AF.Copy,
                            scale=scale[:rw, r : r + 1],
                        )
                    nc.scalar.dma_start(out=om_d[gi][r0 : r0 + rw, :], in_=so[:rw, :])

            # ---- kv groups: batch-parallel GEMMs with core-local norms ----
            def kv_group(gi):
                kvx, kvw = kv_tiles[gi]
                pcs = [
                    psum_pool.tile([128, 512], F32, tag="ps", name=f"pkv_{gi}_{ci}")
                    for ci in range(N_CHUNKS)
                ]
                for k in range(5):
                    for ci in range(N_CHUNKS):
                        nc.tensor.matmul(
                            pcs[ci][:ROWS_PC, :],
                            kvx[:, k, :],
                            kvw[:, k, ci * 512 : (ci + 1) * 512],
                            start=(k == 0),
                            stop=(k == 4),
                        )
                kvo = kv_pool.tile([128, D_MODEL], BF16, tag="kvo", name=f"kvo_{gi}")
                kvssq = small_pool.tile([128, N_CHUNKS], F32, tag=f"kvssq{gi}", name=f"kvssq_{gi}")
                for ci in range(N_CHUNKS):
                    sl = slice(ci * 512, (ci + 1) * 512)
                    kscr = kv_pool.tile([128, 512], BF16, tag="kscr", name=f"kscr_{gi}_{ci}")
                    nc.vector.tensor_copy(kvo[:ROWS_PC, sl], pcs[ci][:ROWS_PC, :])
                    nc.vector.tensor_mul(kscr[:ROWS_PC, :], pcs[ci][:ROWS_PC, :], kvo[:ROWS_PC, sl])
                    nc.vector.reduce_sum(
                        kvssq[:ROWS_PC, ci : ci + 1], kscr[:ROWS_PC, :], axis=mybir.AxisListType.X
                    )
                kvs = small_pool.tile([128, 1], F32, tag=f"kvs{gi}", name=f"kvs_{gi}")
                nc.vector.reduce_sum(kvs[:ROWS_PC, :], kvssq[:ROWS_PC, :], axis=mybir.AxisListType.X)
                nc.scalar.sqrt(kvs[:ROWS_PC, :], kvs[:ROWS_PC, :])
                kvsc = small_pool.tile([128, 1], F32, tag=f"kvsc{gi}", name=f"kvsc_{gi}")
                nc.vector.reciprocal(kvsc[:ROWS_PC, :], kvs[:ROWS_PC, :])
                nc.vector.tensor_scalar_mul(kvo[:ROWS_PC, :], kvo[:ROWS_PC, :], kvsc[:ROWS_PC, :])
                nc.sync.dma_start(out=kvo_d[gi][:, :], in_=kvo[:ROWS_PC, :])

            # ---- schedule ----
            identity_load(0)
            gemm_group(0)          # Wup
            identity_compute(0)
            identity_load(1)
            identity_compute(1)
            gemm_group(1)          # Wgate
            kv_load(0)
            kv_load(1)
            identity_load(2)
            identity_load(3)
            finish_group(0)        # AllReduce A overlaps Wgate/Wdown
            identity_load(4)
            identity_load(5)
            gemm_group(2)          # Wdown
            finish_group(1)        # AllReduce B overlaps Wdown
            kv_group(0)            # tail work: covers AllReduce C latency
            kv_group(1)
            identity_compute(2)
            identity_compute(3)
            identity_compute(4)
            identity_compute(5)
            finish_group(2)        # AllReduce C + final scale/store

    nc.compile()
    return nc


_NC = None


def _get_nc():
    global _NC
    if _NC is None:
        _NC = build_program()
    return _NC


def _prep_inputs(lora_tokens, weights):
    """Host-side sharding: gather token groups, transpose contraction onto
    partitions, slice weights per core, bf16-ify."""
    lora = np.ascontiguousarray(lora_tokens)

    def pack_supertiles(arr_t, kb):
        # [K, C] -> [K/(128*kb), 128, kb*C]: dense per-partition runs so each
        # super-tile DMA is one fully-contiguous block.
        K, C = arr_t.shape
        nsup = K // (128 * kb)
        return np.ascontiguousarray(
            arr_t.reshape(nsup, kb, 128, C).transpose(0, 2, 1, 3).reshape(nsup, 128, kb * C)
        )

    def pack_kv(arr_t):
        # [640, C] -> [128, 5*C]
        K, C = arr_t.shape
        return np.ascontiguousarray(
            arr_t.reshape(5, 128, C).transpose(1, 0, 2).reshape(128, 5 * C)
        )

    shared = {}
    for gi, (off, wname) in enumerate(BIG_GROUPS):
        pos = _positions(off)
        x = lora[:, pos, :].reshape(ROWS, BIG_IND)
        shared[f"xt_{gi}"] = pack_supertiles(x.T.astype(NP_BF16), KB_BIG)
    kv_x = {}
    for gi, (off, wname) in enumerate(KV_GROUPS):
        pos = _positions(offset=off)
        kv_x[gi] = lora[:, pos, :KV_IND].reshape(ROWS, KV_IND)
        shared[f"kvw_{gi}"] = pack_kv(weights[wname].T.astype(NP_BF16))

    id_pos = np.sort(np.concatenate([_positions(o) for o in IDENTITY_OFFSETS]))
    in_maps = []
    bpc = B // N_CORES
    for c in range(N_CORES):
        m = dict(shared)
        for gi, (off, wname) in enumerate(BIG_GROUPS):
            wsl = weights[wname][c * D_SHARD : (c + 1) * D_SHARD, :]  # [320, 10240]
            m[f"wt_{gi}"] = pack_supertiles(wsl.T.astype(NP_BF16), KB_BIG)
        for gi in range(len(KV_GROUPS)):
            m[f"kvx_{gi}"] = pack_kv(
                kv_x[gi][c * ROWS_PC : (c + 1) * ROWS_PC, :].T.astype(NP_BF16)
            )
        m["id_x"] = np.ascontiguousarray(
            lora[c * bpc : (c + 1) * bpc, :, :][:, id_pos, :D_MODEL]
        ).reshape(ID_ROWS, D_MODEL).astype(NP_BF16)
        in_maps.append(m)
    return in_maps, id_pos


def run(inputs, trace=False):
    nc = _get_nc()
    weights = {k: inputs[k] for k in ("Wk", "Wv", "Wgate", "Wup", "Wdown")}
    in_maps, id_pos = _prep_inputs(inputs["lora_tokens"], weights)
    res = run_bass_kernel_spmd(nc, in_maps, CORE_IDS, trace=trace)

    out = np.zeros((B, NUM_LAYERS * TOKENS_PER_LAYER, D_MODEL), dtype=np.float32)
    bpc = B // N_CORES
    for c in range(N_CORES):
        r = res.results[c]
        out[c * bpc : (c + 1) * bpc, id_pos, :] = (
            r["out_id"].astype(np.float32).reshape(bpc, len(id_pos), D_MODEL)
        )
        for gi, (off, wname) in enumerate(BIG_GROUPS):
            pos = _positions(off)
            out[:, pos, c * D_SHARD : (c + 1) * D_SHARD] = (
                r[f"om_{gi}"].astype(np.float32).reshape(B, NUM_LAYERS, D_SHARD)
            )
        for gi, (off, wname) in enumerate(KV_GROUPS):
            pos = _positions(off)
            out[c * bpc : (c + 1) * bpc, pos, :] = (
                r[f"kvo_{gi}"].astype(np.float32).reshape(bpc, NUM_LAYERS, D_MODEL)
            )
    return out, res


def kernel(**inputs) -> np.ndarray:
    out, _ = run(inputs, trace=False)
    return out
